# revision 17
# baseline (speedup 1.0000x reference)
"""Diagonal-matrix multiply kernel for Trainium2: y = x * |diagonal_|.

Full input x is (65536, 1024) f32; diagonal_ is (1024,) f32.
Data-parallel across 8 NeuronCores: each core processes 8192 contiguous
rows of x; the diagonal is replicated to every core.

Per-core kernel (raw bass, cumulative per-engine semaphores):
  - broadcast-DMA diagonal_ into a [128, 1024] SBUF tile, |d| once on
    the vector engine.
  - stream tiles of [128 partitions x R rows x 1024 cols] per pass:
    BF16 loads (the host converts x to bf16 before upload — the
    rel-err budget is 2e-2, bf16 in + bf16 out costs ~8e-3, and this
    problem is HBM-bound: bf16 both ways moves 32 MiB per core instead
    of 48); DVE multiplies each tile against |d| writing bf16; bf16
    stores. Buffer slots pipeline load/multiply/store; loads gate on
    the multiply that freed their slot (not the store), so the read
    stream runs ahead.
  - DMA queue assignment is mode-selectable: "a" = loads on the SP
    HWDGE ring, stores on the ACT HWDGE ring; "ah" = same with |d|
    cast to bf16 (2x DVE mode); "c" = loads AND stores round-robined
    over SP / ACT / Pool(SWDGE) to test the per-ring throughput cap.
  - kernel() returns float32 (host converts the bf16 device output).

Measured findings (interleaved K-differencing, machine-load drift of
+-15% between runs): mode "c" (SWDGE third queue) is 1.7x WORSE —
software DGE sustains only ~80 GB/s and throttles every third tile;
"ah" is within noise of "a" (DVE is never the bottleneck). Tile
geometry is flat within noise for r in {2,4,8} x bufs in {5..12};
r=4/bufs=6 was distinctly best (69 us) in the cleanest run. Default
config streams 32 MiB per core per pass at 440-515 GB/s/core
(65-80 us) vs the f32-input version's 48 MiB at ~344 (146-155 us).
"""

from contextlib import ExitStack

import numpy as np
import jax
import jax.numpy as jnp
from jax.sharding import Mesh, NamedSharding, PartitionSpec
from jax.experimental.shard_map import shard_map

import concourse.bass as bass
from concourse import mybir
from concourse.bass2jax import (
    _bass_exec_p,
    install_neuronx_cc_hook,
    partition_id_tensor,
)

N_CORES = 8
ROWS, COLS = 65536, 1024
SHARD = ROWS // N_CORES  # 8192 rows per core
P = 128
R = 4                    # consecutive rows per partition line
NTILES = SHARD // (P * R)
BUFS = 6                 # in-flight slots: xt bf16 + yt bf16 per slot
MODE = "a"


def _build_probe(reps: int, r_rows: int, bufs: int, mode: str) -> bass.Bass:
    """Timing probes (y is NOT the correct product): "ld" = loads only,
    "st" = stores only (garbage SBUF), "ldst" = load->store passthrough
    (y = x) with the same slot pipeline but no compute. Comparing these
    against mode "a" under identical machine load decomposes the
    bottleneck into per-ring / HBM-total / compute-chain components."""
    R_, BUFS_ = r_rows, bufs
    FREE_ = R_ * COLS
    NTILES_ = SHARD // (P * R_)
    nc = bass.Bass()
    x = nc.dram_tensor("x", [SHARD, COLS], mybir.dt.bfloat16, kind="ExternalInput")
    nc.dram_tensor("diagonal_", [COLS], mybir.dt.float32, kind="ExternalInput")
    y = nc.dram_tensor("y", [SHARD, COLS], mybir.dt.bfloat16, kind="ExternalOutput")
    xv = x[:].rearrange("(n p r) m -> n p (r m)", p=P, r=R_)
    yv = y[:].rearrange("(n p r) m -> n p (r m)", p=P, r=R_)
    total = reps * NTILES_

    with ExitStack() as ctx:
        xt = ctx.enter_context(
            nc.sbuf_tensor([P, BUFS_, FREE_], mybir.dt.bfloat16)
        )
        lds = ctx.enter_context(nc.semaphore("lds"))
        sts = ctx.enter_context(nc.semaphore("sts"))
        block = ctx.enter_context(nc.Block(no_gpsimd_drain=True))

        if mode in ("ld", "ldst"):
            @block.sync
            def _(eng):
                for t in range(total):
                    n, s = t % NTILES_, t % BUFS_
                    if t >= BUFS_:
                        gate = lds if mode == "ld" else sts
                        eng.wait_ge(gate, 16 * (t - BUFS_ + 1))
                    eng.dma_start(out=xt[:, s, :], in_=xv[n]).then_inc(lds, 16)

        if mode in ("st", "ldst"):
            @block.scalar
            def _(eng):
                for t in range(total):
                    n, s = t % NTILES_, t % BUFS_
                    if mode == "ldst":
                        eng.wait_ge(lds, 16 * (t + 1))
                    elif t >= BUFS_:
                        eng.wait_ge(sts, 16 * (t - BUFS_ + 1))
                    eng.dma_start(out=yv[n], in_=xt[:, s, :]).then_inc(sts, 16)

        if mode == "ld2":
            # loads split across BOTH rings by tile parity: does a single
            # direction scale past one ring's throughput?
            lds2 = ctx.enter_context(nc.semaphore("lds2"))

            def half(eng_dec, par, sem):
                @eng_dec
                def _(eng):
                    i = 0
                    for t in range(total):
                        if t % 2 != par:
                            continue
                        n, s = t % NTILES_, t % BUFS_
                        if i >= BUFS_ // 2:
                            eng.wait_ge(sem, 16 * (i - BUFS_ // 2 + 1))
                        eng.dma_start(out=xt[:, s, :], in_=xv[n]).then_inc(
                            sem, 16
                        )
                        i += 1
            half(block.sync, 0, lds)
            half(block.scalar, 1, lds2)

    return nc


def _build_phase(reps: int, r_rows: int, G: int, mode: str) -> bass.Bass:
    """Phase-segregated schedule: HBM reads and writes NEVER overlap
    (measured: concurrent R/W streams run at 383 GB/s combined vs
    557/505 GB/s for pure-direction streams). Tiles are processed in
    batches of G: [load G tiles] -> [store G tiles], strictly
    alternating on the HBM; DVE multiplies (SBUF-only, in-place) hide
    inside the phases. mode "p" = loads on SP ring / stores on ACT;
    mode "p2" = BOTH rings carry each phase (tile parity split)."""
    R_ = r_rows
    FREE_ = R_ * COLS
    NTILES_ = SHARD // (P * R_)
    total = reps * NTILES_
    assert total % G == 0 and G % 2 == 0
    halfG = G // 2
    nc = bass.Bass()
    x = nc.dram_tensor("x", [SHARD, COLS], mybir.dt.bfloat16, kind="ExternalInput")
    d = nc.dram_tensor("diagonal_", [COLS], mybir.dt.float32, kind="ExternalInput")
    y = nc.dram_tensor("y", [SHARD, COLS], mybir.dt.bfloat16, kind="ExternalOutput")
    xv = x[:].rearrange("(n p r) m -> n p (r m)", p=P, r=R_)
    yv = y[:].rearrange("(n p r) m -> n p (r m)", p=P, r=R_)
    d_ap = d[:]
    d_bcast = bass.AP(
        tensor=d_ap.tensor, offset=d_ap.offset, ap=[[0, P], d_ap.ap[0]]
    )

    with ExitStack() as ctx:
        draw = ctx.enter_context(nc.sbuf_tensor([P, COLS], mybir.dt.float32))
        absd = ctx.enter_context(nc.sbuf_tensor([P, COLS], mybir.dt.float32))
        xt = ctx.enter_context(
            nc.sbuf_tensor([P, G, FREE_], mybir.dt.bfloat16)
        )
        dsem = ctx.enter_context(nc.semaphore("d_sem"))
        vs = ctx.enter_context(nc.semaphore("vs_sem"))
        lds_sp = ctx.enter_context(nc.semaphore("lds_sp"))
        lds_act = ctx.enter_context(nc.semaphore("lds_act"))
        sts_sp = ctx.enter_context(nc.semaphore("sts_sp"))
        sts_act = ctx.enter_context(nc.semaphore("sts_act"))
        block = ctx.enter_context(nc.Block(no_gpsimd_drain=True))

        dmul3 = absd[:, None, :].broadcast_to((P, R_, COLS))

        def ring_body(e):
            """e = 0 -> SP(sync), 1 -> ACT(scalar)."""
            def body(eng):
                if e == 1:
                    eng.dma_start(out=draw[:], in_=d_bcast).then_inc(dsem, 16)
                my_lds = (lds_sp, lds_act)[e]
                my_sts = (sts_sp, sts_act)[e]
                # per batch: ALL of this engine's loads, THEN all its
                # stores (a store's batch-gate waits on loads that must
                # precede it in program order)
                for b in range(total // G):
                    first_load = True
                    for t in range(b * G, (b + 1) * G):
                        if not ((mode == "p" and e == 0)
                                or (mode == "p2" and t % 2 == e)):
                            continue
                        if first_load and b >= 1:
                            # batch b loads only after ALL batch b-1 stores
                            if mode == "p":
                                eng.wait_ge(sts_act, 16 * (b * G))
                            else:
                                eng.wait_ge(sts_sp, 16 * (b * halfG))
                                eng.wait_ge(sts_act, 16 * (b * halfG))
                        first_load = False
                        n, s = t % NTILES_, t % G
                        eng.dma_start(out=xt[:, s, :], in_=xv[n]).then_inc(
                            my_lds, 16
                        )
                    first_store = True
                    for t in range(b * G, (b + 1) * G):
                        if not ((mode == "p" and e == 1)
                                or (mode == "p2" and t % 2 == e)):
                            continue
                        if first_store:
                            # batch b stores only after ALL batch b loads
                            if mode == "p":
                                eng.wait_ge(lds_sp, 16 * ((b + 1) * G))
                            else:
                                eng.wait_ge(lds_sp, 16 * ((b + 1) * halfG))
                                eng.wait_ge(lds_act, 16 * ((b + 1) * halfG))
                        first_store = False
                        n, s = t % NTILES_, t % G
                        eng.wait_ge(vs, t + 2)  # multiply t done
                        eng.dma_start(out=yv[n], in_=xt[:, s, :]).then_inc(
                            my_sts, 16
                        )
            return body

        block.sync(ring_body(0))

        @block.vector
        def _(vector):
            vector.wait_ge(dsem, 16)
            vector.scalar_tensor_tensor(
                out=absd[:], in0=draw[:], scalar=-1.0, in1=draw[:],
                op0=mybir.AluOpType.mult, op1=mybir.AluOpType.max,
            ).then_inc(vs, 1)
            vector.wait_ge(vs, 1)
            for t in range(total):
                s = t % G
                if mode == "p" or t % 2 == 0:
                    vector.wait_ge(
                        lds_sp, 16 * ((t + 1) if mode == "p" else (t // 2 + 1))
                    )
                else:
                    vector.wait_ge(lds_act, 16 * (t // 2 + 1))
                x3 = xt[:, s, :].rearrange("p (r m) -> p r m", r=R_)
                vector.tensor_mul(x3, x3, dmul3).then_inc(vs, 1)

        block.scalar(ring_body(1))

    return nc


def _build_nc(reps: int = 1, r_rows: int = R, bufs: int = BUFS,
              mode: str = MODE) -> bass.Bass:
    if mode in ("ld", "st", "ldst", "ld2"):
        return _build_probe(reps, r_rows, bufs, mode)
    if mode in ("p", "p2"):
        return _build_phase(reps, r_rows, bufs, mode)
    R_, BUFS_ = r_rows, bufs
    FREE_ = R_ * COLS
    NTILES_ = SHARD // (P * R_)
    use_dh = mode in ("ah", "c")     # bf16 |d| -> DVE 2x mode
    setup = 2 if use_dh else 1       # vs increments before first multiply
    nc = bass.Bass()
    x = nc.dram_tensor("x", [SHARD, COLS], mybir.dt.bfloat16, kind="ExternalInput")
    d = nc.dram_tensor("diagonal_", [COLS], mybir.dt.float32, kind="ExternalInput")
    y = nc.dram_tensor("y", [SHARD, COLS], mybir.dt.bfloat16, kind="ExternalOutput")

    xv = x[:].rearrange("(n p r) m -> n p (r m)", p=P, r=R_)
    yv = y[:].rearrange("(n p r) m -> n p (r m)", p=P, r=R_)

    d_ap = d[:]
    d_bcast = bass.AP(
        tensor=d_ap.tensor,
        offset=d_ap.offset,
        ap=[[0, P], d_ap.ap[0]],
    )
    total = reps * NTILES_

    # queue assignment per tile index: (load_engine, store_engine)
    # engines: 0 = SP(sync), 1 = ACT(scalar), 2 = Pool(gpsimd SWDGE)
    if mode in ("a", "ah"):
        load_eng = lambda t: 0
        store_eng = lambda t: 1
    elif mode == "c":
        load_eng = lambda t: (0, 1, 2)[t % 3]
        store_eng = lambda t: (1, 2, 0)[t % 3]
    else:
        raise ValueError(mode)

    with ExitStack() as ctx:
        draw = ctx.enter_context(nc.sbuf_tensor([P, COLS], mybir.dt.float32))
        absd = ctx.enter_context(nc.sbuf_tensor([P, COLS], mybir.dt.float32))
        absdh = ctx.enter_context(nc.sbuf_tensor([P, COLS], mybir.dt.bfloat16))
        xt = ctx.enter_context(
            nc.sbuf_tensor([P, BUFS_, FREE_], mybir.dt.bfloat16)
        )
        yt = ctx.enter_context(
            nc.sbuf_tensor([P, BUFS_, FREE_], mybir.dt.bfloat16)
        )
        dsem = ctx.enter_context(nc.semaphore("d_sem"))
        vs = ctx.enter_context(nc.semaphore("vs_sem"))
        ld_sems = [
            ctx.enter_context(nc.semaphore(f"ld_sem{i}")) for i in range(BUFS_)
        ]
        st_sems = [
            ctx.enter_context(nc.semaphore(f"st_sem{i}")) for i in range(BUFS_)
        ]
        block = ctx.enter_context(nc.Block(no_gpsimd_drain=(mode != "c")))

        dmul = absdh if use_dh else absd
        dmul3 = dmul[:, None, :].broadcast_to((P, R_, COLS))

        def dma_body(eng_idx):
            """Per-engine instruction stream: for each tile, enqueue its
            load and/or store if assigned to this engine."""
            def body(eng):
                # d broadcast rides the ACT ring so x loads start instantly
                if eng_idx == 1:
                    eng.dma_start(out=draw[:], in_=d_bcast).then_inc(dsem, 16)
                for t in range(total):
                    n, s = t % NTILES_, t % BUFS_
                    if load_eng(t) == eng_idx:
                        if t >= BUFS_:
                            # xt slot s is free once the multiply that read it ran
                            eng.wait_ge(vs, setup + 1 + (t - BUFS_))
                        eng.dma_start(out=xt[:, s, :], in_=xv[n]).then_inc(
                            ld_sems[s], 16
                        )
                    if store_eng(t) == eng_idx:
                        eng.wait_ge(vs, t + setup + 1)
                        eng.dma_start(out=yv[n], in_=yt[:, s, :]).then_inc(
                            st_sems[s], 16
                        )
            return body

        block.sync(dma_body(0))

        @block.vector
        def _(vector):
            vector.wait_ge(dsem, 16)
            # |d| = max(d * -1, d) in one DVE op
            vector.scalar_tensor_tensor(
                out=absd[:], in0=draw[:], scalar=-1.0, in1=draw[:],
                op0=mybir.AluOpType.mult, op1=mybir.AluOpType.max,
            ).then_inc(vs, 1)
            vector.wait_ge(vs, 1)
            if use_dh:
                vector.tensor_scalar_mul(
                    out=absdh[:], in0=absd[:], scalar1=1.0
                ).then_inc(vs, 1)
                vector.wait_ge(vs, 2)
            for t in range(total):
                s, cyc = t % BUFS_, t // BUFS_
                vector.wait_ge(ld_sems[s], 16 * (cyc + 1))
                if cyc > 0:
                    # yt slot s is free once its previous store drained
                    vector.wait_ge(st_sems[s], 16 * cyc)
                x3 = xt[:, s, :].rearrange("p (r m) -> p r m", r=R_)
                y3 = yt[:, s, :].rearrange("p (r m) -> p r m", r=R_)
                vector.tensor_mul(y3, x3, dmul3).then_inc(vs, 1)

        block.scalar(dma_body(1))
        if any(load_eng(t) == 2 or store_eng(t) == 2 for t in range(total)):
            block.gpsimd(dma_body(2))

    return nc


class _Runner:
    def __init__(self, reps: int = 1, r_rows: int = R, bufs: int = BUFS,
                 mode: str = MODE):
        install_neuronx_cc_hook()
        self.nc = _build_nc(reps, r_rows, bufs, mode)
        nc = self.nc
        assert nc.dbg_addr is None

        in_names = ["x", "diagonal_"]
        out_names = ["y"]
        out_avals = [jax.core.ShapedArray((SHARD, COLS), jnp.bfloat16)]
        all_names = in_names + out_names
        partition_name = (
            nc.partition_id_tensor.name if nc.partition_id_tensor else None
        )
        if partition_name is not None:
            all_names = all_names + [partition_name]

        def _body(*args):
            operands = list(args)
            if partition_name is not None:
                operands.append(partition_id_tensor())
            return tuple(
                _bass_exec_p.bind(
                    *operands,
                    out_avals=tuple(out_avals),
                    in_names=tuple(all_names),
                    out_names=tuple(out_names),
                    lowering_input_output_aliases=(),
                    sim_require_finite=True,
                    sim_require_nnan=True,
                    nc=nc,
                )
            )

        devices = jax.devices()[:N_CORES]
        assert len(devices) == N_CORES
        self.mesh = Mesh(np.asarray(devices), ("core",))
        spec = PartitionSpec("core")
        self.sharding = NamedSharding(self.mesh, spec)
        n_args = len(in_names) + len(out_names)
        self.fn = jax.jit(
            shard_map(
                _body,
                mesh=self.mesh,
                in_specs=(spec,) * n_args,
                out_specs=(spec,) * len(out_names),
                check_rep=False,
            ),
            donate_argnums=(2,),
            keep_unused=True,
        )

    def out_buf(self):
        if getattr(self, "_buf", None) is None:
            self._buf = jax.jit(
                lambda: jnp.zeros((ROWS, COLS), jnp.bfloat16),
                out_shardings=self.sharding,
            )()
        return self._buf

    def __call__(self, x_global, d_global, buf):
        return self.fn(x_global, d_global, buf)[0]


_RUNNERS: dict[tuple, _Runner] = {}


def _get_runner(reps: int = 1, r_rows: int = R, bufs: int = BUFS,
                mode: str = MODE) -> _Runner:
    key = (reps, r_rows, bufs, mode)
    if key not in _RUNNERS:
        _RUNNERS[key] = _Runner(reps, r_rows, bufs, mode)
    return _RUNNERS[key]


def kernel(x: np.ndarray, diagonal_: np.ndarray) -> np.ndarray:
    import ml_dtypes

    r = _get_runner(1)
    x = np.ascontiguousarray(x, dtype=np.float32).astype(ml_dtypes.bfloat16)
    diagonal_ = np.ascontiguousarray(diagonal_, dtype=np.float32)
    d_global = np.tile(diagonal_, N_CORES)
    y = r(x, d_global, r.out_buf())
    r._buf = y
    return np.asarray(y).astype(np.float32)
